# revision 139
# baseline (speedup 1.0000x reference)
"""Trainium2 Bass kernel for windowed (sparse) attention with memory KV.

Sequence-sharded across 8 NeuronCores: core c computes output tokens
[c*512, (c+1)*512) for both batches and all heads, with a 1-window (128
token) k/v halo. The full attn_bias is never shipped: only the block-
diagonal and sub-diagonal 128x128 blocks each core needs (pre-transposed,
mask folded in as -inf rows, duplicated for the two heads of a head-pair).

Engine plan (cost-model driven):
  PE:  all matmuls run with a bf16 moving operand => 1 cyc/row at ANY
       free size (fp32r needs free>=256), so the edge sim chunks are
       trimmed to their 128 live q columns; the bias rides PE as bf16
       identity matmuls accumulated into the sim psum (cheapest engine
       for it by far, and ACT/DVE pace the attention phase).
  DVE: kT stage-2 adds, recip, out scale, outT copies, y adds.
  ACT: qT bias+copy, exp (one strided op per (hp,kc) covering both
       heads), emem exp, kT/v psum->sbuf bf16 copies, w0/w1 y copies.
  Batch-0 kT accumulates in two 3-chunk stages (bf16 partial in SBUF)
  so psum banks rotate while the wk/xT DMA stream is still arriving;
  mem-key sim+exp for all head pairs is hoisted into the proj phase
  (ps2 banks + ACT both have slack there). exp tile ring (60) decouples
  the ACT exp stream from PV consumption. y leaves as bf16.

  Sim psum: both head-halves of a kc chunk share ONE bank (cols 0:256 /
  256:512) with strictly SEQUENTIAL accumulation groups (a bank must
  never hold two open groups — interleaved groups fail birsim). The
  same 4 banks then give a sim ring of 4 instead of 2, which halves the
  ring-latency critical path (ring2 x exp-latency was pacing the whole
  attention phase ~670ns/hp above PE).
"""

import numpy as np

B, N, DIM = 2, 4096, 768
H, DH = 16, 64
W = 128
DI = H * DH                 # 1024
NEG = -3.4028235e38
BNEG = -1.0e30          # masked-bias value: exp() underflows to 0, finite
NCORES = 8
TOK = N // NCORES           # 512
NWIN = TOK // W             # 4
KTOK = TOK + W              # 640
NKC = KTOK // W             # 5
KC6 = DIM // 128            # 6 contraction chunks over DIM
DC8 = DI // 128             # 8 chunks over DI

# engine-assignment knobs (tuned against TimelineSim)
ENG_V_COPY = "act"          # psum -> v_ext bf16
ENG_OUTT_COPY = "dve"       # psum -> outT
ENG_KT_COPY = "act"         # psum -> kT



def build_bass():
    import concourse.mybir as mybir
    import concourse.tile as tile
    from concourse import bacc
    from concourse.masks import make_identity
    from contextlib import ExitStack

    f32 = mybir.dt.float32
    bf16 = mybir.dt.bfloat16
    Exp = mybir.ActivationFunctionType.Exp
    Identity = mybir.ActivationFunctionType.Identity

    nc = bacc.Bacc("TRN2")

    # xkvT: feature-major x with halo, [B*768, 640]
    xkvT_d = nc.dram_tensor("xkvT", [DIM, W + 2 * TOK], bf16,
                            kind="ExternalInput")
    biasc_d = nc.dram_tensor("biasc", [B * NKC * W, 2 * W], bf16,
                             kind="ExternalInput")
    wq_d = nc.dram_tensor("wq", [DIM, DI], bf16, kind="ExternalInput")
    bqs_d = nc.dram_tensor("bqs", [DC8, 128], f32, kind="ExternalInput")
    wkv_d = nc.dram_tensor("wkv", [DIM, 2 * DI], bf16,
                             kind="ExternalInput")
    wo_d = nc.dram_tensor("wo", [DI, DIM], bf16, kind="ExternalInput")
    memk_d = nc.dram_tensor("memk", [128, 8 * DC8], bf16,
                            kind="ExternalInput")
    memv_d = nc.dram_tensor("memv", [8, 16 * 65], bf16, kind="ExternalInput")
    y_d = nc.dram_tensor("y", [2 * TOK, DIM], bf16, kind="ExternalOutput")

    with ExitStack() as ctx:
        tc = ctx.enter_context(tile.TileContext(nc))
        const_p = ctx.enter_context(tc.tile_pool(name="const", bufs=1))
        w_p = ctx.enter_context(tc.tile_pool(name="w", bufs=2 * KC6))
        wo_p = ctx.enter_context(tc.tile_pool(name="wo", bufs=DC8))
        xt_p = ctx.enter_context(tc.tile_pool(name="xt", bufs=6))
        oa_p = ctx.enter_context(tc.tile_pool(name="oa", bufs=4))
        kt_p = ctx.enter_context(tc.tile_pool(name="kt", bufs=2 * DC8))
        qt_p = ctx.enter_context(tc.tile_pool(name="qt", bufs=2 * DC8))
        v_p = ctx.enter_context(tc.tile_pool(name="v", bufs=NKC))
        exp_p = ctx.enter_context(tc.tile_pool(name="exp", bufs=45))
        em_p = ctx.enter_context(tc.tile_pool(name="em", bufs=2 * DC8))
        ot_p = ctx.enter_context(tc.tile_pool(name="ot", bufs=4 * DC8))
        y_p = ctx.enter_context(tc.tile_pool(name="y", bufs=6))
        rc_p = ctx.enter_context(tc.tile_pool(name="rc", bufs=8))
        ps_p = ctx.enter_context(tc.tile_pool(name="ps", bufs=4, space="PSUM"))
        ps2_p = ctx.enter_context(tc.tile_pool(name="ps2", bufs=4,
                                               space="PSUM"))

        def pstile(shape):
            return ps_p.tile(shape, f32, tag="ps", name="ps",
                             padded_shape=[128, 512])

        def psvtile():
            return pstile([128, 65])

        def pstile2():
            # single-bank sim tile: halves at cols 0:256 / 256:512 with
            # sequential accumulation groups — same 4 banks give a ring
            # of 4, halving the sim-ring critical path
            return ps2_p.tile([128, 512], f32, tag="ps2", name="ps2",
                              padded_shape=[128, 512])

        ident = const_p.tile([128, 128], f32)
        make_identity(nc, ident)
        identb = const_p.tile([128, 128], bf16)
        nc.vector.tensor_copy(identb, ident)

        bias_sb = const_p.tile([W, B * NKC * 256], bf16)
        memk_sb = const_p.tile([128, 8 * DC8], bf16)
        memv_sb = const_p.tile([8, 16 * 65], bf16)
        bqs_sb = const_p.tile([128, DC8], f32)
        wo_sb = [wo_p.tile([128, DIM], bf16, tag="wo", name=f"wo{_}")
                 for _ in range(DC8)]

        def load_consts():
            # one [128, B*NKC*256] bf16 HWDGE transfer: rides the sync queue
            # BEHIND the critical wk/xT stream instead of stealing early DMA
            # slots via gpsimd SWDGE descriptor generation
            nc.sync.dma_start(
                bias_sb.rearrange("p (c q) -> p c q", q=256),
                biasc_d.rearrange("(c p) q -> p c q", p=W))
            nc.sync.dma_start(memk_sb, memk_d[:, :])
            nc.sync.dma_start(memv_sb, memv_d[:, :])

        def load_wo(ds):
            for d in ds:
                nc.sync.dma_start(wo_sb[d], wo_d[d * 128:(d + 1) * 128, :])

        nc.gpsimd.dma_start(bqs_sb, bqs_d.rearrange("c p -> p c"))
        wqs = [w_p.tile([128, DI], bf16, tag="wq", name=f"wq{_}", bufs=KC6)
               for _ in range(KC6)]

        def copy_eng(eng):
            return nc.scalar.copy if eng == "act" else nc.vector.tensor_copy

        # wk/wv stay resident across batches: allocate once (distinct tags
        # so the per-tag rings never recycle a live slot), DMA once
        wk = [w_p.tile([128, DI], bf16, tag="wk", name=f"wk{_}", bufs=KC6)
              for _ in range(KC6)]
        wv_all = w_p.tile([128, KC6 * DI], bf16, tag="wv", name="wv",
                          bufs=1)
        wv = [wv_all[:, k * DI:(k + 1) * DI] for k in range(KC6)]

        prev_kT, prev_v4 = None, None
        for b in range(B):
            # ---- wk/xT chunk-interleaved so the first kT chain paces with
            # chunk arrival instead of waiting for the whole stream ----
            # pass 1 computes kv only for its OWN 512 tokens: the halo
            # window's kT/v come from pass 0's still-resident tiles
            xw = KTOK if b == 0 else TOK
            xT = [xt_p.tile([128, 1024], bf16, tag="xtoa", name=f"xt{_}",
                            bufs=6)[:, :xw] for _ in range(KC6)]
            for d in range(KC6):
                r0 = d * 128
                if b == 0 and d == 0:
                    # split the first chunks so the first kT matmul can
                    # start as soon as its 128-col slice lands
                    nc.sync.dma_start(wk[0][:, 0:128],
                                      wkv_d[0:128, 0:128])
                    nc.sync.dma_start(xT[0][:, 0:320],
                                      xkvT_d[r0:r0 + 128, 0:320])
                    nc.sync.dma_start(wk[0][:, 128:DI],
                                      wkv_d[0:128, 128:DI])
                    nc.sync.dma_start(xT[0][:, 320:KTOK],
                                      xkvT_d[r0:r0 + 128, 320:KTOK])
                    continue
                if b == 0:
                    nc.sync.dma_start(wk[d], wkv_d[d * 128:(d + 1) * 128, :DI])
                    nc.sync.dma_start(xT[d], xkvT_d[r0:r0 + 128, :KTOK])
                else:
                    nc.sync.dma_start(xT[d],
                                      xkvT_d[r0:r0 + 128, KTOK:KTOK + TOK])
            if b == 0:
                for d in range(KC6):
                    nc.sync.dma_start(wqs[d], wq_d[d * 128:(d + 1) * 128, :])
                nc.sync.dma_start(
                    wv_all.rearrange("p (c d) -> p c d", d=DI),
                    wkv_d[:, DI:].rearrange("(c p) d -> p c d", p=128))
                load_consts()
            kT = [kt_p.tile([128, KTOK], bf16, tag="kt", name=f"kt{_}")
                  for _ in range(DC8)]

            def kt_banks(d8):
                # odd d8 chains ride the (otherwise idle) ps2 banks so 8
                # accumulation chains stay in flight while the wk/xT DMA
                # stream trickles in
                if d8 % 2:
                    return [pstile2()[:, 0:320], pstile2()[:, 0:320]]
                return [pstile([128, 320]), pstile([128, 320])]

            def kt_chain(d8, nt, ps, k6s, first):
                for k6 in k6s:
                    nc.tensor.matmul(
                        ps, wk[k6][:, d8 * 128:(d8 + 1) * 128],
                        xT[k6][:, nt * 320:(nt + 1) * 320],
                        start=(k6 == k6s[0]), stop=(k6 == k6s[-1]))
                dst = kT[d8][:, nt * 320:(nt + 1) * 320]
                if first:
                    copy_eng(ENG_KT_COPY)(dst, ps)
                else:
                    nc.vector.tensor_add(dst, dst, ps)

            if b == 0:
                # two 3-chunk stages with a bf16 partial: banks rotate at
                # the stage boundary, so all 16 chains advance chunk-major
                # while the wk/xT stream is still arriving
                for stage, k6s in enumerate(([0, 1, 2], [3, 4, 5])):
                    for d8 in range(DC8):
                        ps_nt = kt_banks(d8)
                        for nt in range(2):
                            kt_chain(d8, nt, ps_nt[nt], k6s, stage == 0)
            else:
                # own 512 tokens only: cols 128:640, sourced from xT1[0:512]
                for d8 in range(DC8):
                    ps_nt = kt_banks(d8)
                    for nt in range(2):
                        ps = ps_nt[nt][:, 0:256]
                        for k6 in range(KC6):
                            nc.tensor.matmul(
                                ps, wk[k6][:, d8 * 128:(d8 + 1) * 128],
                                xT[k6][:, nt * 256:(nt + 1) * 256],
                                start=(k6 == 0), stop=(k6 == KC6 - 1))
                        copy_eng(ENG_KT_COPY)(
                            kT[d8][:, W + nt * 256:W + (nt + 1) * 256], ps)

            # ---- qT = (Wq*s).T @ xT + bq*s ----
            qT = [qt_p.tile([128, TOK], bf16, tag="qt", name=f"qt{_}")
                  for _ in range(DC8)]
            for d8 in range(DC8):
                ps = pstile([128, 512])
                qoff = W if b == 0 else 0
                for k6 in range(KC6):
                    nc.tensor.matmul(
                        ps, wqs[k6][:, d8 * 128:(d8 + 1) * 128],
                        xT[k6][:, qoff:qoff + TOK],
                        start=(k6 == 0), stop=(k6 == KC6 - 1))
                nc.scalar.activation(qT[d8], ps, Identity,
                                     bias=bqs_sb[:, d8:d8 + 1])

            # ---- v = xT.T @ Wv (token-major bf16, 65-strided + ones col) ----
            tt0 = 0 if b == 0 else 1
            v_new = [v_p.tile([128, 16 * 65], bf16, tag="v", name=f"v{_}")
                     for _ in range(NKC - tt0)]
            v_ext = v_new if b == 0 else [prev_v4] + v_new
            def do_vproj():
                for tt in range(tt0, NKC):
                    v3 = v_ext[tt].rearrange("p (h c) -> p h c", c=65)
                    nc.gpsimd.memset(v3[:, :, 64:65], 1.0)
                    xcol = tt * 128 - (0 if b == 0 else 128)
                    for half in range(2):
                        ps = pstile([128, 512])
                        for k6 in range(KC6):
                            nc.tensor.matmul(
                                ps, xT[k6][:, xcol:xcol + 128],
                                wv[k6][:, half * 512:(half + 1) * 512],
                                start=(k6 == 0), stop=(k6 == KC6 - 1))
                        copy_eng("act" if half == 0 else ENG_V_COPY)(
                            v3[:, half * 8:(half + 1) * 8, 0:64],
                            ps.rearrange("p (h c) -> p h c", c=64))

            # ---- attention: software-pipelined sim(hp+1) ahead of PV(hp) ----
            out_all = [oa_p.tile([128, 1024], bf16, tag="oa", name=f"oa{_}")
                       for _ in range(NWIN)]
            outT = [[ot_p.tile([128, 128], bf16, tag="ot", name=f"ot{w}_{d}")
                     for d in range(DC8)] for w in range(NWIN)]
            qlo_of = lambda kc: min(max(0, (kc - 1) * W), TOK - 2 * W)

            # mem-key sim+exp for all head pairs up front: depends only on
            # qT, and both the psm psum (ps2 ring, idle during proj) and
            # the ACT exps land in the proj phase where they have slack —
            # the attention phase then paces on PE instead of ACT
            emems = []
            for hp2 in range(0, DC8, 2):
                for h in range(2):
                    hp = hp2 + h
                    psm = pstile2()[:8, :]
                    nc.tensor.matmul(
                        psm, memk_sb[:, hp * 8:(hp + 1) * 8], qT[hp],
                        start=True, stop=True)
                    emem = em_p.tile([8, 512], bf16, tag="em", name="em")
                    nc.scalar.activation(emem, psm, Exp)
                    emems.append(emem)

            def sim_chunks(hp, exp_tiles, out_state):
                # generator: one yield per emitted sim kc chunk, so the
                # caller can interleave PV work between chunks (PE executes
                # its queue in order; a chunk stalled on the 2-deep psum
                # ring must not block already-ready PV matmuls behind it)
                out_state[0] = emems[hp]
                for kc in range(NKC):
                    qlo = qlo_of(kc)
                    bcol = (b * NKC + kc) * 256
                    bias = bias_sb[:, bcol:bcol + 256]  # heads share the bias
                    psp = pstile2()
                    # edge chunks serve a single window: only 128 of the 256
                    # q columns are live, so sim/bias/exp cover just those
                    # (PV reads only the live quarters); bf16 operands keep
                    # the PE at 1 cyc/row even at 128 free
                    edge = kc in (0, NKC - 1)
                    lo, live = (128, 128) if kc == NKC - 1 else \
                        (0, 128 if edge else 256)
                    # strictly sequential accumulation groups: half A
                    # opens and closes before half B opens, so the shared
                    # bank never holds two open groups
                    kthp = prev_kT[hp] if (b == 1 and kc == 0) else kT[hp]
                    kcol = 4 * W if (b == 1 and kc == 0) else kc * W
                    nc.tensor.matmul(
                        psp[:, lo:lo + live],
                        kthp[0:64, kcol:kcol + W],
                        qT[hp][0:64, qlo + lo:qlo + lo + live],
                        start=True, stop=False)
                    nc.tensor.matmul(psp[:, lo:lo + live], identb,
                                     bias[:, lo:lo + live],
                                     start=False, stop=True)
                    nc.tensor.matmul(
                        psp[:, 256 + lo:256 + lo + live],
                        kthp[64:128, kcol:kcol + W],
                        qT[hp][64:128, qlo + lo:qlo + lo + live],
                        start=True, stop=False)
                    nc.tensor.matmul(psp[:, 256 + lo:256 + lo + live], identb,
                                     bias[:, lo:lo + live],
                                     start=False, stop=True)
                    pv = psp.rearrange("p (h q) -> p h q", q=256)[:, :,
                                                                  lo:lo + live]
                    eb = exp_p.tile([128, 512], bf16, tag="expb", name="expb")
                    ebv = eb.rearrange("p (h q) -> p h q", q=256)[:, :,
                                                                  lo:lo + live]
                    nc.scalar.activation(ebv, pv, Exp)
                    exp_tiles[kc] = eb
                    yield

            def pv_pairs(hp, emem, exp_tiles):
                # generator: one yield per 2-group PV chunk
                allgroups = [(w, h01) for w in range(NWIN) for h01 in range(2)]
                for gi in range(0, 8, 2):
                    groups = allgroups[gi:gi + 2]
                    psvs = {}
                    for w, h01 in groups:
                        hg = 2 * hp + h01
                        psvs[(w, h01)] = psvtile()
                        nc.tensor.matmul(
                            psvs[(w, h01)],
                            emem[:, w * W:(w + 1) * W],
                            memv_sb[:, hg * 65:(hg + 1) * 65],
                            start=True, stop=False)
                    for w, h01 in groups:
                        hg = 2 * hp + h01
                        pcol = h01 * 256 + w * W - qlo_of(w)
                        nc.tensor.matmul(
                            psvs[(w, h01)], exp_tiles[w][:, pcol:pcol + W],
                            v_ext[w].rearrange("p (h c) -> p h c", c=65)[:, hg],
                            start=False, stop=False)
                    for w, h01 in groups:
                        hg = 2 * hp + h01
                        ccol = h01 * 256 + w * W - qlo_of(w + 1)
                        nc.tensor.matmul(
                            psvs[(w, h01)],
                            exp_tiles[w + 1][:, ccol:ccol + W],
                            v_ext[w + 1].rearrange("p (h c) -> p h c", c=65)[:, hg],
                            start=False, stop=True)
                    for w, h01 in groups:
                        hg = 2 * hp + h01
                        psv = psvs[(w, h01)]
                        rc = rc_p.tile([128, 1], f32, tag="rc", name="rc")
                        nc.vector.reciprocal(rc, psv[:, 64:65])
                        nc.vector.tensor_scalar_mul(
                            out_all[w][:, hg * 64:(hg + 1) * 64],
                            psv[:, 0:64], rc)
                    yield

            def drain(gen):
                if gen is not None:
                    for _ in gen:
                        pass

            def do_transpose(hp):
                # transpose this head-pair's 128-wide slab of each window
                # (DMA-xbar transposes were tried here: the per-DMA HWDGE
                # queue occupancy starves the batch-1 weight stream)
                for w in range(NWIN):
                    ps = pstile([128, 64]).bitcast(bf16)
                    nc.tensor.transpose(
                        ps, out_all[w][:, hp * 128:(hp + 1) * 128], identb)
                    copy_eng("act" if hp >= 6 else ENG_OUTT_COPY)(
                        outT[w][hp], ps)

            # first-half output projection (head-pairs 0..3), injectable
            # into late attention slots once pv(3) has filled outT[w][0:4]
            ysb_half = {}

            def oproj_half1(w):
                ysb = y_p.tile([128, DIM], f32, tag="y", name="y", bufs=4)
                ysb_half[w] = ysb
                for nn in range(2):
                    ps = pstile([128, 384])
                    for d8 in range(6):
                        nc.tensor.matmul(
                            ps, outT[w][d8],
                            wo_sb[d8][:, nn * 384:(nn + 1) * 384],
                            start=(d8 == 0), stop=(d8 == 5))
                    nc.vector.tensor_copy(ysb[:, nn * 384:(nn + 1) * 384], ps)

            state = {}
            exp0, em0 = {}, [None]
            drain(sim_chunks(0, exp0, em0))
            state[0] = (em0[0], exp0)
            do_vproj()          # v chains overlap sim(0)'s exp latency
            for hp in range(1, DC8 + 1):
                if b == 0 and 2 <= hp <= 5:
                    load_wo(range((hp - 2) * 2, (hp - 1) * 2))
                pgen = pv_pairs(hp - 1, *state.pop(hp - 1))
                if hp < DC8:
                    expn, emn = {}, [None]
                    sgen = sim_chunks(hp, expn, emn)
                    # interleave: s s p s p s p s p — a sim chunk stalled
                    # on its psum-ring slot never blocks ready PV work
                    next(sgen)
                    for _ in range(4):
                        next(sgen, None)
                        next(pgen)
                    drain(sgen)
                    drain(pgen)
                    state[hp] = (emn[0], expn)
                else:
                    drain(pgen)
                do_transpose(hp - 1)
                if hp >= 7:
                    oproj_half1(hp - 5)

            # ---- finish: pre-split windows first — their cheap 2-chunk
            # adds release y DMAs early, draining the DMA queue while the
            # full-oproj windows still run on PE ----
            for w in (2, 3, 0, 1):
                if w in ysb_half:
                    ysb = ysb_half[w]
                    yb = y_p.tile([128, DIM], bf16, tag="yb", name="yb", bufs=2)
                    for nn in range(2):
                        ps = pstile([128, 384])
                        for d8 in range(6, DC8):
                            nc.tensor.matmul(
                                ps, outT[w][d8],
                                wo_sb[d8][:, nn * 384:(nn + 1) * 384],
                                start=(d8 == 6), stop=(d8 == DC8 - 1))
                        nc.vector.tensor_add(
                            yb[:, nn * 384:(nn + 1) * 384],
                            ysb[:, nn * 384:(nn + 1) * 384], ps)
                        nc.sync.dma_start(
                            y_d[b * TOK + w * W:b * TOK + (w + 1) * W,
                                nn * 384:(nn + 1) * 384],
                            yb[:, nn * 384:(nn + 1) * 384])
                    continue
                else:
                    ysb = y_p.tile([128, DIM], bf16, tag="y", name="y", bufs=4)
                    for nn in range(2):
                        ps = pstile([128, 384])
                        for d8 in range(DC8):
                            nc.tensor.matmul(
                                ps, outT[w][d8],
                                wo_sb[d8][:, nn * 384:(nn + 1) * 384],
                                start=(d8 == 0), stop=(d8 == DC8 - 1))
                        nc.vector.tensor_copy(
                            ysb[:, nn * 384:(nn + 1) * 384], ps)
                        nc.sync.dma_start(
                            y_d[b * TOK + w * W:b * TOK + (w + 1) * W,
                                nn * 384:(nn + 1) * 384],
                            ysb[:, nn * 384:(nn + 1) * 384])
            prev_kT, prev_v4 = kT, v_ext[NKC - 1]
    nc.compile()
    return nc


def host_prep(x, mask, attn_bias, Wq, bq, Wkv, Wo, memory_kv):
    import ml_dtypes
    bf16 = ml_dtypes.bfloat16
    s = np.float32(DH ** -0.5)
    wq = (np.asarray(Wq, np.float32) * s).astype(bf16)
    bqs = (np.asarray(bq, np.float32) * s).astype(np.float32).reshape(DC8, 128)
    wkv = np.ascontiguousarray(np.asarray(Wkv, np.float32)).astype(bf16)
    wo = np.ascontiguousarray(np.asarray(Wo, np.float32)).astype(bf16)
    x = np.asarray(x, np.float32)
    mask = np.asarray(mask).astype(bool)
    attn_bias = np.asarray(attn_bias, np.float32)
    mk = np.asarray(memory_kv[0], np.float32)
    mv = np.asarray(memory_kv[1], np.float32)

    # block-diagonal mem-key pack: one [128, 8] lhsT slab per head pair
    memk = np.zeros((128, 8 * DC8), np.float32)
    for hp in range(DC8):
        memk[0:64, hp * 8:hp * 8 + 4] = mk[2 * hp].T
        memk[64:128, hp * 8 + 4:hp * 8 + 8] = mk[2 * hp + 1].T
    memk = memk.astype(bf16)
    # memv: [8, 16*65]; per head hg only rows (hg%2)*4:+4 are nonzero, so a
    # full-8-partition matmul against the combined emem picks the right head
    memv = np.zeros((8, 16 * 65), np.float32)
    for hg in range(H):
        r = (hg % 2) * 4
        memv[r:r + 4, hg * 65:hg * 65 + 64] = mv[hg]
        memv[r:r + 4, hg * 65 + 64] = 1.0
    memv = memv.astype(bf16)

    shared = dict(wq=wq, bqs=bqs, wkv=wkv, wo=wo, memk=memk, memv=memv)
    xT_full = np.ascontiguousarray(x.transpose(0, 2, 1))    # [B, 768, 4096]
    in_maps = []
    for c in range(NCORES):
        # core c: batch c//4, tokens [(c%4)*1024, (c%4+1)*1024) in 2 passes
        cb, cw = c // 4, c % 4
        q0 = cw * 2 * TOK
        xkvT = np.zeros((DIM, W + 2 * TOK), np.float32)
        lo = q0 - W
        src_lo = max(lo, 0)
        xkvT[:, src_lo - lo:] = xT_full[cb, :, src_lo:q0 + 2 * TOK]
        biasc = np.full((B, NKC, W, 2 * W), BNEG, np.float32)
        for b in range(B):        # b = pass index within this core's block
            w0 = cw * 2 * NWIN + b * NWIN
            for kc in range(NKC):
                gk = w0 + kc - 1
                if gk < 0:
                    continue
                kr = slice(gk * W, (gk + 1) * W)
                qlo = min(max(0, (kc - 1) * W), TOK - 2 * W)
                if kc >= 1:
                    qr = slice((w0 + kc - 1) * W, (w0 + kc) * W)
                    col = (kc - 1) * W - qlo
                    biasc[b, kc, :, col:col + W] = attn_bias[cb, qr, kr].T
                if kc <= NWIN - 1:
                    qr = slice((w0 + kc) * W, (w0 + kc + 1) * W)
                    col = kc * W - qlo
                    biasc[b, kc, :, col:col + W] = attn_bias[cb, qr, kr].T
                kmask = mask[cb, gk * W:(gk + 1) * W]
                biasc[b, kc, ~kmask, :] = BNEG
        in_maps.append(dict(
            xkvT=np.ascontiguousarray(xkvT).astype(bf16),
            biasc=np.ascontiguousarray(
                biasc.reshape(B * NKC * W, 2 * W)).astype(bf16),
            **shared))
    return in_maps


_CACHE = {}


def kernel(**inputs):
    import sys
    if "/opt/trn_rl_repo" not in sys.path:
        sys.path.insert(0, "/opt/trn_rl_repo")
    from concourse.bass_utils import run_bass_kernel_spmd

    in_maps = host_prep(**inputs)
    if "nc" not in _CACHE:
        _CACHE["nc"] = build_bass()
    nc = _CACHE["nc"]
    res = run_bass_kernel_spmd(nc, in_maps, core_ids=list(range(NCORES)))
    out = np.empty((B, N, DIM), np.float32)
    for c in range(NCORES):
        cb, cw = c // 4, c % 4
        out[cb, cw * 2 * TOK:(cw + 1) * 2 * TOK] = np.asarray(
            res.results[c]["y"], dtype=np.float32).reshape(2 * TOK, DIM)
    return out


if __name__ == "__main__":
    import sys
    sys.path.insert(0, "/opt/trn_rl_repo")
    nc = build_bass()
    print("build OK")

